# revision 14
# baseline (speedup 1.0000x reference)
"""Trainium2 Bass kernel for causal linear ("cumulative") attention.

Math (matches the reference nn.Module):
    q  = x @ Wq.T + bq                      [T,B,H*K]
    k  = LN(x @ Wk.T + bk) * k_gamma + k_beta    [T,B,K]
    v  = LN(x @ Wv.T + bv) * v_gamma + v_beta    [T,B,E]
    qn[t] = mean_h softmax(q[t,h,:])        [T,B,K]
    S_t   = sum_{s<=t} k_s v_s^T            [K,E]  (never materialized per-t)
    attn[t] = (qn[t]/sqrt(t+1)) @ S_t       [T,B,E]

Sharding: T is split into 8 contiguous blocks of 256 rows, one per
NeuronCore (both batches on every core).

Launch 1 (per core): fused QKV projection GEMM (streamed behind the
W DMA), softmax-mean, layernorms, plus the ENTIRE block-local causal
attention (chunked: masked diagonal scores + intra-block state carry),
emitting attn1 = local attention term, qnr^T, and the block state
S_local.

Host glue (free): exclusive prefix-sum of the 8 S_local tensors.

Launch 2 (tiny): term2 = qnr @ S_prefix  (rank-64 GEMM per core).

Host: attn = attn1 + term2 (elementwise gather/add), cast fp32.
"""

import shutil

import ml_dtypes
import numpy as np

import concourse.bass as bass
import concourse.tile as tile
from concourse import mybir, bacc
from concourse.bass_utils import run_bass_kernel_spmd

F32 = mybir.dt.float32
F32R = mybir.dt.float32r
BF16 = mybir.dt.bfloat16
I32 = mybir.dt.int32
AF = mybir.ActivationFunctionType
ALU = mybir.AluOpType
AX = mybir.AxisListType

T, B, E, H, K = 2048, 2, 1024, 16, 64
NCORES = 8
RPC = T // NCORES          # 256 rows (per batch) per core
P = 128
NE = E // P                # 8 contraction chunks
NTT = 4                    # row-tiles per core: tt = 2*b + c
EPS = 1e-5
MAGIC = 0x5F3759DF

# fused projection column order: [q 1024 | v 1024 | k 64]
NCOLS = 2112
QOFF, VOFF, KOFF = 0, 1024, 2048

TRACE = False          # test.py flips this for profiling runs
LAST_EXEC_NS = []      # exec_time_ns of each launch when TRACE

_CACHE = {}


def _rsqrt(nc, pool, var_ap, magic_sb, n):
    """rstd[p, n] = 1/sqrt(var + eps) on the vector engine (bit trick)."""
    eng = nc.vector
    a = pool.tile([P, n], F32, tag="rsq_a")
    eng.tensor_scalar_add(a[:], var_ap, EPS)
    y = pool.tile([P, n], F32, tag="rsq_y")
    yi = y[:].bitcast(I32)
    eng.tensor_scalar(yi, a[:].bitcast(I32), 1, None, ALU.arith_shift_right)
    eng.tensor_tensor(yi, magic_sb[:, :n], yi, ALU.subtract)
    t = pool.tile([P, n], F32, tag="rsq_t")
    for _ in range(2):
        eng.tensor_tensor(t[:], y[:], y[:], ALU.mult)
        eng.tensor_tensor(t[:], t[:], a[:], ALU.mult)
        eng.tensor_scalar(t[:], t[:], -0.5, 1.5, ALU.mult, ALU.add)
        eng.tensor_tensor(y[:], y[:], t[:], ALU.mult)
    return y


def _build_l1(trivial: bool):
    """Projection GEMM + softmax-mean + layernorms + local causal attn."""
    nc = bacc.Bacc("TRN2", target_bir_lowering=False, debug=False,
                   num_devices=NCORES)
    NEB = NE if trivial else NE + 1   # extra ones-chunk carries the biases
    xT = nc.dram_tensor("xT", [P, NEB, 2 * RPC], BF16,
                        kind="ExternalInput").ap()
    WT = nc.dram_tensor("WT", [P, NEB, NCOLS], BF16,
                        kind="ExternalInput").ap()
    rs = nc.dram_tensor("rs", [2 * RPC], F32, kind="ExternalInput").ap()
    triu = nc.dram_tensor("triu", [P, P], F32, kind="ExternalInput").ap()
    if not trivial:
        vgam = nc.dram_tensor("vgam", [E], F32, kind="ExternalInput").ap()
        vbet = nc.dram_tensor("vbet", [E], F32, kind="ExternalInput").ap()
        kgam = nc.dram_tensor("kgam", [K], F32, kind="ExternalInput").ap()
        kbet = nc.dram_tensor("kbet", [K], F32, kind="ExternalInput").ap()

    # attn1: local-attention term, [b, rows 0:256, E]
    attn1_o = nc.dram_tensor("attn1", [B, RPC, E], BF16,
                             kind="ExternalOutput").ap()
    # qnrT: partitions (b*64+feat), free = rows 0:256
    qnrT_o = nc.dram_tensor("qnrT", [P, RPC], BF16,
                            kind="ExternalOutput").ap()
    # S_local: partitions (b*64+feat), free = E
    S_o = nc.dram_tensor("S", [P, E], F32, kind="ExternalOutput").ap()

    with tile.TileContext(nc) as tc:
        with (
            tc.tile_pool(name="big", bufs=1) as big,
            tc.tile_pool(name="work", bufs=2) as work,
            tc.tile_pool(name="small", bufs=4) as small,
        ):
            # ---------------- constants + input DMA ----------------
            xt_sb = big.tile([P, NEB, 2 * RPC], BF16)
            wt_sb = big.tile([P, NEB, NCOLS], BF16)
            # x rides the idle SWDGE queue; W streams e-ordered on the two
            # HWDGE rings (chunks e and e+1 in flight together), with the
            # first two chunks split in halves for an early compute start.
            nc.gpsimd.dma_start(xt_sb[:, :NEB // 2, :], xT[:, :NEB // 2, :])
            nc.gpsimd.dma_start(xt_sb[:, NEB // 2:, :], xT[:, NEB // 2:, :])
            HC = NCOLS // 2
            for e in (0, 1):
                eng = nc.sync if e == 0 else nc.scalar
                eng.dma_start(wt_sb[:, e, :HC], WT[:, e, :HC])
                eng.dma_start(wt_sb[:, e, HC:], WT[:, e, HC:])
            for e in range(2, NEB):
                eng = nc.sync if e % 2 == 0 else nc.scalar
                eng.dma_start(wt_sb[:, e, :], WT[:, e, :])

            rs_sb = big.tile([P, NTT], F32)
            nc.gpsimd.dma_start(rs_sb[:], rs.rearrange("(a p) -> p a", p=P))
            triu_sb = big.tile([P, P], F32)
            nc.gpsimd.dma_start(triu_sb[:], triu[:])
            magic_sb = big.tile([P, 4], I32)
            nc.vector.memset(magic_sb[:], MAGIC)
            if not trivial:
                vg_sb = big.tile([P, E], F32)
                nc.scalar.dma_start(vg_sb[:], vgam[None, :].partition_broadcast(P))
                vb_sb = big.tile([P, E], F32)
                nc.scalar.dma_start(vb_sb[:], vbet[None, :].partition_broadcast(P))
                kg_sb = big.tile([P, K], F32)
                nc.scalar.dma_start(kg_sb[:], kgam[None, :].partition_broadcast(P))
                kb_sb = big.tile([P, K], F32)
                nc.scalar.dma_start(kb_sb[:], kbet[None, :].partition_broadcast(P))

            warm = big.tile([P, P], BF16)
            nc.vector.memset(warm[:], 0.0)

            # persistent SBUF results of phase A/B
            exp_sb = big.tile([P, NTT, H * K], BF16)     # exp(q)
            v_sb = big.tile([P, NTT, E], BF16)           # v raw -> v_ln
            kr_sb = big.tile([P, NTT, K], F32)           # k raw
            k_ln = big.tile([P, 2, B, K], BF16)          # (c, b, feat)
            qnr_sb = big.tile([P, 2, B, K], BF16)        # (c, b, feat)
            kT_sb = big.tile([P, 2, P], BF16)            # (b*64+f, c, pos)
            qnrT_sb = big.tile([P, 2, P], BF16)
            en_sb = big.tile([P, H, K], BF16)

            mvs = [None] * NTT   # [128, 2, 2] (k,v) x (mean, var)
            rstd = [None] * 2    # per pair: [128, 4]

            # ---------------- phase A: streamed GEMM ----------------
            ctx_a = tc.tile_pool(name="ps_q", bufs=2, space="PSUM")
            ps_q = ctx_a.__enter__()
            ctx_av = tc.tile_pool(name="ps_v", bufs=1, space="PSUM")
            ps_v = ctx_av.__enter__()
            ctx_ak = tc.tile_pool(name="ps_k", bufs=1, space="PSUM")
            ps_k = ctx_ak.__enter__()
            warm_ps = ps_k.tile([P, 64], F32, tag="k", name="warm_ps")
            for w in range(32):
                nc.tensor.matmul(warm_ps[:], warm[:], warm[:, :64],
                                 start=True, stop=True)
            for tt in range(NTT):
                b, c = tt % 2, tt // 2
                lb = 2 * b + c           # position in the row layout
                rows = bass.ts(lb, P)
                q_ps = ps_q.tile([P, 2, 512], F32, tag="q", name=f"q_{tt}")
                v_ps = ps_v.tile([P, 2, 512], F32, tag="v", name=f"v_{tt}")
                k_ps = ps_k.tile([P, K], F32, tag="k", name=f"k_{tt}")
                for e in range(NEB):
                    st = (e == 0)
                    sp = (e == NEB - 1)
                    lhs = xt_sb[:, e, rows]
                    for h in range(2):
                        nc.tensor.matmul(q_ps[:, h, :], lhs,
                                         wt_sb[:, e, QOFF + 512 * h:
                                               QOFF + 512 * (h + 1)],
                                         start=st, stop=sp)
                    for h in range(2):
                        nc.tensor.matmul(v_ps[:, h, :], lhs,
                                         wt_sb[:, e, VOFF + 512 * h:
                                               VOFF + 512 * (h + 1)],
                                         start=st, stop=sp)
                    nc.tensor.matmul(k_ps[:], lhs,
                                     wt_sb[:, e, KOFF:KOFF + K],
                                     start=st, stop=sp)

                # drain: exp(q) on scalar; v copy + stats, k copy on vector
                nc.scalar.activation(exp_sb[:, tt, :512], q_ps[:, 0, :],
                                     AF.Exp)
                nc.scalar.activation(exp_sb[:, tt, 512:], q_ps[:, 1, :],
                                     AF.Exp)
                nc.vector.tensor_scalar_mul(
                    v_sb[:, tt, :].rearrange("p (a b) -> p a b", a=2),
                    v_ps[:], 1.0)
                nc.vector.tensor_copy(kr_sb[:, tt, :], k_ps[:])
                vst = small.tile([P, 2, 6], F32, tag="vst")
                nc.vector.bn_stats(vst[:, 0, :], v_sb[:, tt, :512])
                nc.vector.bn_stats(vst[:, 1, :], v_sb[:, tt, 512:])
                kst = small.tile([P, 6], F32, tag="kst")
                nc.vector.bn_stats(kst[:], kr_sb[:, tt, :])
                m = small.tile([P, 2, 2], F32, tag="mvs", name=f"mvs_{tt}")
                nc.vector.bn_aggr(m[:, 0, :], kst[:])
                nc.vector.bn_aggr(m[:, 1, :], vst[:])
                mvs[tt] = m

                # ------- deferred per-pair + per-tt elementwise chain ------
                # pair boundary: tts (0,1) -> after tt1's stats; (2,3) after.
                if tt % 2 == 1:
                    pr = tt // 2         # pair index == chunk c
                    var4 = small.tile([P, 4], F32, tag="var4")
                    nc.vector.tensor_copy(var4[:, 0:2], mvs[tt - 1][:, :, 1])
                    nc.vector.tensor_copy(var4[:, 2:4], mvs[tt][:, :, 1])
                    rstd[pr] = _rsqrt(nc, small, var4[:], magic_sb, 4)
                    for i, t2 in enumerate((tt - 1, tt)):
                        b, c = t2 % 2, t2 // 2
                        rst = rstd[pr]
                        # k layernorm
                        nc.vector.tensor_scalar(
                            k_ln[:, c, b, :], kr_sb[:, t2, :],
                            mvs[t2][:, 0, 0:1], rst[:, 2 * i:2 * i + 1],
                            ALU.subtract, ALU.mult)
                        if not trivial:
                            nc.vector.tensor_tensor(
                                k_ln[:, c, b, :], k_ln[:, c, b, :],
                                kg_sb[:], ALU.mult)
                            nc.vector.tensor_tensor(
                                k_ln[:, c, b, :], k_ln[:, c, b, :],
                                kb_sb[:], ALU.add)
                        # v layernorm (in place on the bf16 copy)
                        nc.vector.tensor_scalar(
                            v_sb[:, t2, :], v_sb[:, t2, :],
                            mvs[t2][:, 1, 0:1], rst[:, 2 * i + 1:2 * i + 2],
                            ALU.subtract, ALU.mult)
                        if not trivial:
                            nc.vector.tensor_tensor(
                                v_sb[:, t2, :], v_sb[:, t2, :],
                                vg_sb[:], ALU.mult)
                            nc.vector.tensor_tensor(
                                v_sb[:, t2, :], v_sb[:, t2, :],
                                vb_sb[:], ALU.add)
                        # softmax-mean -> qnr
                        ex = exp_sb[:, t2, :].rearrange(
                            "p (h k) -> p h k", h=H)
                        gs = small.tile([P, H], F32, tag="gs")
                        nc.vector.reduce_sum(gs[:], ex, axis=AX.X)
                        gr = small.tile([P, H], F32, tag="gr")
                        nc.vector.reciprocal(gr[:], gs[:])
                        nc.vector.tensor_tensor(
                            en_sb[:], ex,
                            gr[:, :, None].to_broadcast((P, H, K)), ALU.mult)
                        for width in (8, 4, 2, 1):
                            nc.vector.tensor_tensor(
                                en_sb[:, :width, :], en_sb[:, :width, :],
                                en_sb[:, width:2 * width, :], ALU.add)
                        lb2 = 2 * b + c
                        nc.vector.tensor_scalar_mul(
                            qnr_sb[:, c, b, :], en_sb[:, 0, :],
                            rs_sb[:, lb2:lb2 + 1])
                    # transposes for this chunk (both batches now done)
                    c = pr
                    nc.sync.dma_start(
                        kT_sb[:, c, :],
                        k_ln[:, c, :, :].rearrange("p b k -> p (b k)"),
                        transpose=True)
                    nc.scalar.dma_start(
                        qnrT_sb[:, c, :],
                        qnr_sb[:, c, :, :].rearrange("p b k -> p (b k)"),
                        transpose=True)
                    if tt == NTT - 1:
                        nc.gpsimd.dma_start(
                            qnrT_o[:],
                            qnrT_sb[:].rearrange("p c r -> p (c r)"))

            # ---------------- phase C: local causal attention -------------
            ctx_ak.__exit__(None, None, None)
            ctx_av.__exit__(None, None, None)
            ctx_a.__exit__(None, None, None)
            ctx_sc = tc.tile_pool(name="ps_sc", bufs=2, space="PSUM")
            ps_sc = ctx_sc.__enter__()
            ctx_at = tc.tile_pool(name="ps_at", bufs=2, space="PSUM")
            ps_at = ctx_at.__enter__()
            ctx_s = tc.tile_pool(name="ps_s", bufs=1, space="PSUM")
            ps_s = ctx_s.__enter__()
            Sc0_sb = big.tile([P, E], BF16)
            S_all = ps_s.tile([P, 2, 512], F32, tag="s", name="S_all")
            for c in range(2):
                for b in range(B):
                    bs = slice(64 * b, 64 * (b + 1))
                    sc_ps = ps_sc.tile([P, P], F32, tag="sc",
                                       name=f"scp_{c}_{b}")
                    nc.tensor.matmul(sc_ps[:], kT_sb[bs, c, :],
                                     qnrT_sb[bs, c, :],
                                     start=True, stop=True,
                                     tile_position=(64 * b, 0))
                    sc_sb = work.tile([P, P], BF16, tag="scsb",
                                      name=f"sc_{c}_{b}")
                    nc.vector.tensor_tensor(sc_sb[:], sc_ps[:], triu_sb[:],
                                            ALU.mult)
                    # attention output for (b, c)
                    t2 = 2 * c + b
                    at_ps = ps_at.tile([P, 2, 512], F32, tag="at",
                                       name=f"at_{c}_{b}")
                    for h in range(2):
                        hs = slice(512 * h, 512 * (h + 1))
                        nc.tensor.matmul(at_ps[:, h, :], sc_sb[:],
                                         v_sb[:, t2, hs],
                                         start=True, stop=(c == 0))
                        if c == 1:
                            nc.tensor.matmul(at_ps[:, h, :],
                                             qnrT_sb[bs, 1, :],
                                             Sc0_sb[bs, hs],
                                             start=False, stop=True,
                                             tile_position=(64 * b, 0))
                    at_sb = work.tile([P, E], BF16, tag="atsb",
                                      name=f"atsb_{c}_{b}")
                    if b == 0:
                        nc.vector.tensor_scalar_mul(
                            at_sb[:].rearrange("p (a b) -> p a b", a=2),
                            at_ps[:], 1.0)
                    else:
                        nc.scalar.activation(
                            at_sb[:].rearrange("p (a b) -> p a b", a=2),
                            at_ps[:], AF.Identity)
                    eng = nc.sync if b == 0 else nc.scalar
                    eng.dma_start(attn1_o[b, bass.ts(c, P), :], at_sb[:])

                # block state S_c (both batches via column tiling);
                # accumulate c0+c1 in one psum group, reading the c0
                # partial mid-group for the intra-block carry term.
                for b in range(B):
                    t2 = 2 * c + b
                    for h in range(2):
                        hs = slice(512 * h, 512 * (h + 1))
                        nc.tensor.matmul(S_all[64 * b:64 * (b + 1), h, :],
                                         k_ln[:, c, b, :],
                                         v_sb[:, t2, hs],
                                         start=(c == 0), stop=(c == 1),
                                         tile_position=(0, 64 * b),
                                         skip_group_check=True)
                if c == 0:
                    nc.vector.tensor_scalar_mul(
                        Sc0_sb[:].rearrange("p (a b) -> p a b", a=2),
                        S_all[:], 1.0)

            S_tot = big.tile([P, E], F32)
            nc.scalar.activation(
                S_tot[:].rearrange("p (a b) -> p a b", a=2), S_all[:],
                AF.Identity)
            nc.sync.dma_start(S_o[:], S_tot[:])
            ctx_s.__exit__(None, None, None)
            ctx_at.__exit__(None, None, None)
            ctx_sc.__exit__(None, None, None)

    nc.compile()
    return nc


def _build_l2():
    """term2 = qnr @ S_prefix per core."""
    nc = bacc.Bacc("TRN2", target_bir_lowering=False, debug=False,
                   num_devices=NCORES)
    qnrT = nc.dram_tensor("qnrT", [B, K, RPC], BF16,
                          kind="ExternalInput").ap()
    S0 = nc.dram_tensor("S0", [B, K, E], BF16, kind="ExternalInput").ap()
    t2_o = nc.dram_tensor("t2", [B, RPC, E], BF16,
                          kind="ExternalOutput").ap()

    with tile.TileContext(nc) as tc:
        with (
            tc.tile_pool(name="sg", bufs=1) as sg,
            tc.tile_pool(name="wk", bufs=4) as wk,
            tc.tile_pool(name="psw", bufs=1, space="PSUM") as psw,
            tc.tile_pool(name="ps", bufs=3, space="PSUM") as ps,
        ):
            qn_sb = sg.tile([K, B, RPC], BF16)
            S_sb = sg.tile([K, B, E], BF16)
            nc.sync.dma_start(qn_sb[:], qnrT.rearrange("b k r -> k b r"))
            nc.scalar.dma_start(S_sb[:, 0, :], S0[0])
            nc.sync.dma_start(S_sb[:, 1, :], S0[1])
            warm = sg.tile([P, P], BF16)
            nc.vector.memset(warm[:], 0.0)
            warm_ps = psw.tile([P, 64], F32, tag="warm")
            for w in range(12):
                nc.tensor.matmul(warm_ps[:], warm[:], warm[:, :64],
                                 start=True, stop=True)
            for b in range(B):
                for rt in range(2):
                    at_ps = ps.tile([P, 2, 512], F32, tag="at",
                                    name=f"at_{b}_{rt}")
                    for h in range(2):
                        nc.tensor.matmul(
                            at_ps[:, h, :],
                            qn_sb[:, b, bass.ts(rt, P)],
                            S_sb[:, b, bass.ts(h, 512)],
                            start=True, stop=True)
                    at_sb = wk.tile([P, E], BF16, tag="atsb",
                                    name=f"atsb_{b}_{rt}")
                    if rt == 0:
                        nc.vector.tensor_scalar_mul(
                            at_sb[:].rearrange("p (a b) -> p a b", a=2),
                            at_ps[:], 1.0)
                    else:
                        nc.scalar.activation(
                            at_sb[:].rearrange("p (a b) -> p a b", a=2),
                            at_ps[:], AF.Identity)
                    eng = nc.sync if rt == 0 else nc.scalar
                    eng.dma_start(t2_o[b, bass.ts(rt, P), :], at_sb[:])

    nc.compile()
    return nc


def _get_kernels(trivial: bool):
    key = ("k", trivial)
    if key not in _CACHE:
        _CACHE[key] = (_build_l1(trivial), _build_l2())
    return _CACHE[key]


def kernel(x, attn_mask, Wq, bq, Wk, bk, Wv, bv, k_gamma, k_beta,
           v_gamma, v_beta):
    x = np.ascontiguousarray(np.asarray(x, dtype=np.float32))
    Wq = np.asarray(Wq, dtype=np.float32)
    Wk = np.asarray(Wk, dtype=np.float32)
    Wv = np.asarray(Wv, dtype=np.float32)
    bq = np.asarray(bq, dtype=np.float32)
    bk = np.asarray(bk, dtype=np.float32)
    bv = np.asarray(bv, dtype=np.float32)
    k_gamma = np.asarray(k_gamma, dtype=np.float32)
    k_beta = np.asarray(k_beta, dtype=np.float32)
    v_gamma = np.asarray(v_gamma, dtype=np.float32)
    v_beta = np.asarray(v_beta, dtype=np.float32)

    trivial = (not bq.any() and not bk.any() and not bv.any()
               and not k_beta.any() and not v_beta.any()
               and np.all(k_gamma == 1.0) and np.all(v_gamma == 1.0))
    nc1, nc2 = _get_kernels(trivial)
    del LAST_EXEC_NS[:]
    if TRACE:
        for d in ("/tmp/ktrace_l1", "/tmp/ktrace_l2"):
            shutil.rmtree(d, ignore_errors=True)

    # ---------------- launch 1 ----------------
    NEB = NE if trivial else NE + 1
    WTf = np.concatenate([Wq, Wv, Wk], axis=0).T       # [E, 2112]
    if not trivial:
        bias_row = np.concatenate([bq, bv, bk])[None, :]
        WTf = np.concatenate([WTf, bias_row,
                              np.zeros((P - 1, NCOLS), np.float32)], axis=0)
    WT = np.ascontiguousarray(
        WTf.reshape(NEB, P, NCOLS).transpose(1, 0, 2)).astype(
            ml_dtypes.bfloat16)
    t_idx = np.arange(T, dtype=np.float64)
    rowscale_all = (1.0 / np.sqrt(t_idx + 1.0) / H).astype(np.float32)
    triu = np.triu(np.ones((P, P), dtype=np.float32))

    in1 = []
    for c in range(NCORES):
        rows = slice(c * RPC, (c + 1) * RPC)
        xT_f = np.concatenate([x[rows, 0, :].T, x[rows, 1, :].T], axis=1)
        if not trivial:
            extra = np.zeros((P, 2 * RPC), np.float32)
            extra[0, :] = 1.0
            xT_f = np.concatenate([xT_f, extra], axis=0)
        xT_c = np.ascontiguousarray(
            xT_f.reshape(NEB, P, 2 * RPC).transpose(1, 0, 2)).astype(
                ml_dtypes.bfloat16)
        rs_c = np.ascontiguousarray(
            np.concatenate([rowscale_all[rows]] * 2))
        d = {"xT": xT_c, "WT": WT, "rs": rs_c, "triu": triu}
        if not trivial:
            d.update({"vgam": v_gamma, "vbet": v_beta,
                      "kgam": k_gamma, "kbet": k_beta})
        in1.append(d)
    r1 = run_bass_kernel_spmd(nc1, in1, core_ids=list(range(NCORES)),
                              trace=TRACE,
                              tmpdir="/tmp/ktrace_l1" if TRACE else None)
    if TRACE:
        LAST_EXEC_NS.append(r1.exec_time_ns)

    # ---------------- host glue: prefix-sum of block states -------------
    attn1 = [r1.results[c]["attn1"] for c in range(NCORES)]  # [B,256,E] bf16
    qnrT = [r1.results[c]["qnrT"] for c in range(NCORES)]    # [128,256] bf16
    S_loc = [r1.results[c]["S"] for c in range(NCORES)]      # [128,E] f32

    in2 = []
    acc = np.zeros((P, E), dtype=np.float64)
    for c in range(NCORES):
        S0_c = acc.astype(np.float32).reshape(B, K, E)
        acc += S_loc[c]
        in2.append({"qnrT": np.ascontiguousarray(
                        qnrT[c].reshape(B, K, RPC)).astype(ml_dtypes.bfloat16),
                    "S0": S0_c.astype(ml_dtypes.bfloat16)})
    r2 = run_bass_kernel_spmd(nc2, in2, core_ids=list(range(NCORES)),
                              trace=TRACE,
                              tmpdir="/tmp/ktrace_l2" if TRACE else None)
    if TRACE:
        LAST_EXEC_NS.append(r2.exec_time_ns)

    # ---------------- gather: attn = attn1 + term2 ----------------
    out = np.empty((T, B, E), dtype=np.float32)
    for c in range(NCORES):
        a1 = np.asarray(attn1[c], dtype=np.float32)
        t2 = np.asarray(r2.results[c]["t2"], dtype=np.float32)
        s = a1 + t2
        for b in range(B):
            out[c * RPC:(c + 1) * RPC, b, :] = s[b]
    return out
